# revision 4
# baseline (speedup 1.0000x reference)
"""GAT (nn_GAT_29523605193094) Trainium2 kernel.

The reference keeps the source bug ``src, dst = edges[0], edges[0]``, so the
adjacency matrix is purely diagonal: adj[i, i] = (i appears in edges[0]).
After the -inf masking, row i of the [N, N, H] score tensor has exactly one
finite entry (j = i) when node i is covered, so softmax over axis=1 yields
exactly 1.0 at (i, i), and the output row is exactly h[i] = (X @ W)[i].
Rows for uncovered nodes are all -inf -> softmax is NaN -> NaN output row.
Both cases are reproduced here:

    out = X @ W            (on 8 NeuronCores, row-sharded, bf16 matmul)
    out[~covered] = NaN    (host-side mask from edges[0])

The device work is a row-sharded [4096, 512] @ [512, 256] matmul.  Inputs
are cast to bf16 on the host (the PE runs bf16 at 4x the fp32 rate and the
HBM traffic halves); accumulation is fp32 in PSUM; the output is written
back as bf16 and upcast on the host.  Worst-case relative error vs the
fp32 reference is ~4e-3, well inside the 2e-2 gate.

All DRAM tensors are host-packed into a [128, *] partition-major layout so
every DMA moves 2-4 KiB per partition line: the DMA engines are per-line
overhead bound near 1 KiB lines (the naive [K, M] layout's 512 B - 1 KiB
rows ran at ~220 GB/s; the packed layout sustains the ~410 GB/s core
limit and needs only 5 DMA instructions instead of 12).
"""

import numpy as np

N = 4096
IN = 512
OUT = 256
NCORES = 8
RB = N // NCORES  # 512 rows per core
P = 128
KT = IN // P      # 4 contraction chunks
MT = RB // P      # 4 output row blocks per core

_state = {}

# test.py reads this after a traced call for the HW exec time.
LAST_RESULTS = None


def _build():
    import concourse.mybir as mybir
    import concourse.tile as tile
    from concourse import bacc
    from concourse.bass import ts

    nc = bacc.Bacc(
        "TRN2",
        target_bir_lowering=False,
        debug=False,
        num_devices=NCORES,
    )
    f32 = mybir.dt.float32
    bf16 = mybir.dt.bfloat16
    # Partition-major packing: row p holds K-rows {p, 128+p, 256+p, 384+p}
    # of X_shard^T / W (resp. M-rows of out), so each DMA line is the full
    # per-partition payload (4 KiB / 2 KiB / 2 KiB).
    xt = nc.dram_tensor("xt", [P, KT * RB], bf16, kind="ExternalInput")
    w = nc.dram_tensor("w", [P, KT * OUT], bf16, kind="ExternalInput")
    out = nc.dram_tensor("out", [P, MT * OUT], bf16, kind="ExternalOutput")

    with tile.TileContext(nc) as tc:
        with (
            tc.tile_pool(name="ins", bufs=1) as in_pool,
            tc.tile_pool(name="outs", bufs=1) as out_pool,
            tc.tile_pool(name="ps", bufs=4, space="PSUM") as psum_pool,
        ):
            xt_t = in_pool.tile([P, KT, RB], bf16)
            w_t = in_pool.tile([P, KT, OUT], bf16)
            H = KT // 2
            # w + first xt half in parallel on two queues, then the second
            # xt half; the k-outer matmul loop starts after ~half the input.
            nc.scalar.dma_start(w_t[:, :, :], w[:, :])
            nc.sync.dma_start(xt_t[:, 0:H, :], xt[:, 0 : H * RB])
            nc.sync.dma_start(xt_t[:, H:KT, :], xt[:, H * RB : KT * RB])

            # k-outer / m-inner: when the last k chunk lands only one
            # m-sweep (4 matmuls) remains, minimizing the post-DMA tail.
            pss = [
                psum_pool.tile([P, OUT], f32, name=f"ps{m}", tag="ps")
                for m in range(MT)
            ]
            for k in range(KT):
                for m in range(MT):
                    nc.tensor.matmul(
                        pss[m][:],
                        xt_t[:, k, ts(m, P)],
                        w_t[:, k, :],
                        start=(k == 0),
                        stop=(k == KT - 1),
                    )
            ob = out_pool.tile([P, MT, OUT], bf16)
            for m in range(MT):
                # Alternate PSUM->SBUF downcast between vector and scalar
                # (ACT) so two copies run concurrently; GPSIMD can't read
                # PSUM.  Out DMAs ride queues that are idle by then.
                if m % 2 == 0:
                    nc.vector.tensor_copy(ob[:, m, :], pss[m][:])
                else:
                    nc.scalar.copy(ob[:, m, :], pss[m][:])
                if m == 1:
                    nc.gpsimd.dma_start(out[:, 0 : 2 * OUT], ob[:, 0:2, :])
                elif m == 3:
                    nc.sync.dma_start(out[:, 2 * OUT : 4 * OUT], ob[:, 2:4, :])

    nc.compile()
    return nc


def _pack(a: np.ndarray, chunks: int) -> np.ndarray:
    """[chunks*128, F] row-major -> [128, chunks*F] partition-major."""
    f = a.shape[1]
    return np.ascontiguousarray(
        a.reshape(chunks, P, f).transpose(1, 0, 2).reshape(P, chunks * f)
    )


def kernel(X, edges, W, A):
    global LAST_RESULTS
    import ml_dtypes
    from concourse.bass_utils import run_bass_kernel_spmd

    X = np.asarray(X, dtype=np.float32)
    W = np.asarray(W, dtype=np.float32)
    edges = np.asarray(edges)

    if "nc" not in _state:
        _state["nc"] = _build()
    nc = _state["nc"]

    XT = X.T.astype(ml_dtypes.bfloat16)  # [IN, N]
    wp = _pack(W.astype(ml_dtypes.bfloat16), KT)
    in_maps = [
        {"xt": _pack(XT[:, c * RB : (c + 1) * RB], KT), "w": wp}
        for c in range(NCORES)
    ]
    # The device occasionally reports a transient NRT_EXEC_UNIT_UNRECOVERABLE
    # on an otherwise-good kernel; retry before giving up.
    last_exc = None
    for _attempt in range(3):
        try:
            res = run_bass_kernel_spmd(nc, in_maps, core_ids=list(range(NCORES)))
            break
        except Exception as exc:  # noqa: BLE001
            last_exc = exc
            import time

            time.sleep(2.0)
    else:
        raise last_exc
    LAST_RESULTS = res
    out = np.concatenate(
        [
            res.results[c]["out"]
            .astype(np.float32)
            .reshape(P, MT, OUT)
            .transpose(1, 0, 2)
            .reshape(RB, OUT)
            for c in range(NCORES)
        ],
        axis=0,
    )

    # Reference semantics: nodes absent from edges[0] have an all -inf score
    # row; softmax of that is NaN, which propagates to the output row.
    covered = np.zeros(N, dtype=bool)
    covered[edges[0]] = True
    if not covered.all():
        out[~covered] = np.nan
    return out


# revision 5
# speedup vs baseline: 1.0921x; 1.0921x over previous
"""GAT (nn_GAT_29523605193094) Trainium2 kernel.

The reference keeps the source bug ``src, dst = edges[0], edges[0]``, so the
adjacency matrix is purely diagonal: adj[i, i] = (i appears in edges[0]).
After the -inf masking, row i of the [N, N, H] score tensor has exactly one
finite entry (j = i) when node i is covered, so softmax over axis=1 yields
exactly 1.0 at (i, i), and the output row is exactly h[i] = (X @ W)[i].
Rows for uncovered nodes are all -inf -> softmax is NaN -> NaN output row.
Both cases are reproduced here:

    out = X @ W            (on 8 NeuronCores, row-sharded, bf16 matmul)
    out[~covered] = NaN    (host-side mask from edges[0])

Device-side structure (per core: [512, 512] @ [512, 256] in bf16, fp32
PSUM accumulate, bf16 output, host casts):

- All DRAM tensors are host-packed partition-major ([128, *]) so DMA
  lines are 1-4 KiB (the DMA engines are per-line-overhead bound below
  ~2 KiB lines).
- All input DMAs ride ONE queue, ordered [w, xt_k0..xt_k3], so transfers
  complete in the order the k-outer matmul loop consumes them and the PE
  chases the stream instead of waiting for the slowest transfer.
- The Tensor engine p-state ramps (0.65 -> 1.2 -> 2.4 GHz after ~3 us of
  continuous work), so a short chain of scratch-fed warmup matmuls runs
  during the input stream; the real matmuls then run at full clock.
  Warmup targets the real PSUM banks (safe: the real k0 matmul uses
  start=True, which resets the bank) and reads a memset scratch tile
  (keeping it independent of the streaming input tiles).
"""

import numpy as np

N = 4096
IN = 512
OUT = 256
NCORES = 8
RB = N // NCORES  # 512 rows per core
P = 128
KT = IN // P      # 4 contraction chunks
MT = RB // P      # 4 output row blocks per core
WARMUP = 16       # ~3.4 us of dummy PE work at mid p-state

_state = {}

# test.py reads this after a traced call for the HW exec time.
LAST_RESULTS = None


def _build():
    import concourse.mybir as mybir
    import concourse.tile as tile
    from concourse import bacc
    from concourse.bass import ts

    nc = bacc.Bacc(
        "TRN2",
        target_bir_lowering=False,
        debug=False,
        num_devices=NCORES,
    )
    f32 = mybir.dt.float32
    bf16 = mybir.dt.bfloat16
    # Partition-major packing: row p holds K-rows {p, 128+p, 256+p, 384+p}
    # of X_shard^T / W (resp. M-rows of out), so each DMA line is the full
    # per-partition payload.
    xt = nc.dram_tensor("xt", [P, KT * RB], bf16, kind="ExternalInput")
    w = nc.dram_tensor("w", [P, KT * OUT], bf16, kind="ExternalInput")
    out = nc.dram_tensor("out", [P, MT * OUT], bf16, kind="ExternalOutput")

    with tile.TileContext(nc) as tc:
        with (
            tc.tile_pool(name="ins", bufs=1) as in_pool,
            tc.tile_pool(name="outs", bufs=1) as out_pool,
            tc.tile_pool(name="ps", bufs=4, space="PSUM") as psum_pool,
        ):
            xt_t = in_pool.tile([P, KT, RB], bf16)
            w_t = in_pool.tile([P, KT, OUT], bf16)
            scratch = in_pool.tile([P, OUT], bf16)
            nc.gpsimd.memset(scratch[:], 0)

            nc.sync.dma_start(w_t[:, :, :], w[:, :])
            for k in range(KT):
                nc.sync.dma_start(xt_t[:, k, :], xt[:, k * RB : (k + 1) * RB])

            pss = [
                psum_pool.tile([P, OUT], f32, name=f"ps{m}", tag="ps")
                for m in range(MT)
            ]
            # P-state warmup: keep the PE continuously busy from body start
            # so it reaches full clock by the time real operands land.
            for i in range(WARMUP):
                nc.tensor.matmul(
                    pss[i % MT][:],
                    scratch[:, 0:P],
                    scratch[:],
                    start=True,
                    stop=True,
                )

            # k-outer / m-inner: when the last k chunk lands only one
            # m-sweep (4 matmuls) remains, minimizing the post-DMA tail.
            for k in range(KT):
                for m in range(MT):
                    nc.tensor.matmul(
                        pss[m][:],
                        xt_t[:, k, ts(m, P)],
                        w_t[:, k, :],
                        start=(k == 0),
                        stop=(k == KT - 1),
                    )
            ob = out_pool.tile([P, MT, OUT], bf16)
            for m in range(MT):
                # PSUM->SBUF downcast on vector (GPSIMD can't read PSUM;
                # scalar would pull in a 1.3 us ACT table load that
                # contends with the input stream).
                nc.vector.tensor_copy(ob[:, m, :], pss[m][:])
                if m == 1:
                    nc.scalar.dma_start(out[:, 0 : 2 * OUT], ob[:, 0:2, :])
                elif m == 3:
                    nc.sync.dma_start(out[:, 2 * OUT : 4 * OUT], ob[:, 2:4, :])

    nc.compile()
    return nc


def _pack(a: np.ndarray, chunks: int) -> np.ndarray:
    """[chunks*128, F] row-major -> [128, chunks*F] partition-major."""
    f = a.shape[1]
    return np.ascontiguousarray(
        a.reshape(chunks, P, f).transpose(1, 0, 2).reshape(P, chunks * f)
    )


def kernel(X, edges, W, A):
    global LAST_RESULTS
    import ml_dtypes
    from concourse.bass_utils import run_bass_kernel_spmd

    X = np.asarray(X, dtype=np.float32)
    W = np.asarray(W, dtype=np.float32)
    edges = np.asarray(edges)

    if "nc" not in _state:
        _state["nc"] = _build()
    nc = _state["nc"]

    XT = X.T.astype(ml_dtypes.bfloat16)  # [IN, N]
    wp = _pack(W.astype(ml_dtypes.bfloat16), KT)
    in_maps = [
        {"xt": _pack(XT[:, c * RB : (c + 1) * RB], KT), "w": wp}
        for c in range(NCORES)
    ]
    # The device occasionally reports a transient NRT_EXEC_UNIT_UNRECOVERABLE
    # on an otherwise-good kernel; retry before giving up.
    last_exc = None
    for _attempt in range(3):
        try:
            res = run_bass_kernel_spmd(nc, in_maps, core_ids=list(range(NCORES)))
            break
        except Exception as exc:  # noqa: BLE001
            last_exc = exc
            import time

            time.sleep(2.0)
    else:
        raise last_exc
    LAST_RESULTS = res
    out = np.concatenate(
        [
            res.results[c]["out"]
            .astype(np.float32)
            .reshape(P, MT, OUT)
            .transpose(1, 0, 2)
            .reshape(RB, OUT)
            for c in range(NCORES)
        ],
        axis=0,
    )

    # Reference semantics: nodes absent from edges[0] have an all -inf score
    # row; softmax of that is NaN, which propagates to the output row.
    covered = np.zeros(N, dtype=bool)
    covered[edges[0]] = True
    if not covered.all():
        out[~covered] = np.nan
    return out
